# revision 1
# baseline (speedup 1.0000x reference)
"""Trainium2 Bass kernel for nn_BaseConvPlus (dense_cnn).

Math: the reference computes
  1) kernel[b,c,:,:]  = global-mean of a depthwise 3x3 conv of x          -> [B,CIN,3,3]
  2) win  = einsum(kernel, w_in) + b_in ; wout = einsum(kernel, w_out)
  3) y[b] = conv2d(x[b], weight[b]) with weight[b,o,i] = win[b,i]*wout[b,o]

Identities that make this memory-bound:
  * mean(conv(x, k)) over HxW only needs the total sum, edge-row/col sums
    and corner pixels of each channel (zero 'SAME' padding) - no conv.
    The tap-selection matrix is folded into the host-side wk tables, so
    kernel[b,c,j] = sum_k wkH[c,j,k] * sums[b,c,k] with sums = the 9
    reduced quantities [T, RF, RL, CF, CL, c00, c0L, cL0, cLL].
  * weight[b] is rank-1 across (o, i): y[b,o] = wout[b,o] * z[b] with
    z[b] = sum_i conv2d(x[b,i], win[b,i]).  The conv is done in two PE
    stages (6 image passes instead of 9):
      stage1 (K=128=(b,i), f32r): 3 row-shifted matmuls contract (i, ky)
        -> G[(b,kx), pixels] in PSUM;  DVE-evict to bf16 SBUF
      stage2 (K=12=(b,kx), bf16): 3 col-shifted matmuls contract kx and
        apply wout -> y[(b,o), pixels] directly in PSUM; ACT-evict.

Sharding: pure data parallel, 4 samples per core on 8 cores.
"""
import sys

sys.path.insert(0, "/opt/trn_rl_repo")

from contextlib import ExitStack

import ml_dtypes
import numpy as np

import concourse.bacc as bacc
import concourse.bass as bass
import concourse.mybir as mybir
import concourse.tile as tile
from concourse.bass_utils import run_bass_kernel_spmd

B, CIN, COUT, KS, H, W = 32, 32, 32, 3, 192, 192
NCORES = 8
BC = B // NCORES          # 4 samples per core
P = BC * CIN              # 128 partitions = (sample, channel)
HP = H + 2               # 194 rows (one zero row above and below)
WP = W + 2                # 194: G gets the side padding instead of x
NPIX = HP * W             # 37248: row-padded pixels, rows contiguous
# input chunks (HWDGE fp32 -> SBUF staging; ACT casts to bf16)
CHUNKS = [16] * 11 + [8] * 2
NCHUNK = len(CHUNKS)      # 13
NSTAGE = 4                # staging slots
R = 2                     # output rows per conv tile
NT = H // R               # 96 conv tiles
GT = 8                    # conv tiles per output DMA (16 rows, 1.5 MiB)
NG = NT // GT             # 12 output DMAs
F32 = mybir.dt.float32
F32R = mybir.dt.float32r
BF16 = mybir.dt.bfloat16
AX = mybir.AxisListType
OP = mybir.AluOpType


def build_program(nc: bass.Bass) -> None:
    x_d = nc.dram_tensor("x", [BC, CIN, H, W], F32, kind="ExternalInput").ap()
    wkh_d = nc.dram_tensor("wkh", [P, 81], F32, kind="ExternalInput").ap()
    lwin_d = nc.dram_tensor("lwin", [P, P], BF16, kind="ExternalInput").ap()
    brep_d = nc.dram_tensor("brep", [P, 1], F32, kind="ExternalInput").ap()
    wo9_d = nc.dram_tensor("wo9", [P, 9 * P], BF16, kind="ExternalInput").ap()
    m12_d = nc.dram_tensor("m12", [P, 12], F32, kind="ExternalInput").ap()
    kxm_d = nc.dram_tensor("kxm", [12, 3], F32, kind="ExternalInput").ap()
    ident_d = nc.dram_tensor("ident", [P, P], F32, kind="ExternalInput").ap()
    y_d = nc.dram_tensor("y", [BC, COUT, H, W], F32, kind="ExternalOutput").ap()

    xf = x_d.rearrange("b c h w -> (b c) (h w)")       # [128, 36864]
    yf = y_d.rearrange("b o h w -> (b o) (h w)")       # [128, 36864]

    with tile.TileContext(nc) as tc, ExitStack() as ctx:
        const = ctx.enter_context(tc.tile_pool(name="const", bufs=1))
        ypool = ctx.enter_context(tc.tile_pool(name="ysb", bufs=2))
        psum_g = ctx.enter_context(tc.tile_pool(name="psum_g", bufs=3, space="PSUM"))
        psum_y = ctx.enter_context(tc.tile_pool(name="psum_y", bufs=3, space="PSUM"))
        psum_s = ctx.enter_context(tc.tile_pool(name="psum_s", bufs=1, space="PSUM"))

        xpad = const.tile([P, NPIX], BF16)
        wkh = const.tile([P, 81], F32)
        lwin = const.tile([P, P], BF16)
        brep = const.tile([P, 1], F32)
        wo9 = const.tile([P, 9 * P], BF16)
        m12 = const.tile([P, 12], F32)
        kxm = const.tile([12, 3], F32)
        ident = const.tile([P, P], F32)
        scr = const.tile([P, 16 + 3 * NCHUNK], F32)  # 0:T 1:CF 2:CL 3:RF 4:RL 5..8 corners, then partials
        t81 = const.tile([P, 81], F32)
        kern = const.tile([P, 9], F32)
        kernb = const.tile([P, 9], BF16)
        vout = const.tile([P, 12], F32)
        lkx = const.tile([P, 3 * P], BF16)     # stage2 lhsT per kx (K-padded)
        lky = const.tile([P, 3 * P], BF16)     # stage1 lhsT per ky (M-padded)
        gbuf = const.tile([P, 3 * R * WP], BF16)   # 3 slots of [2 rows x 194]
        stag = const.tile([P, NSTAGE * 16 * W], F32)   # fp32 input staging

        x3 = xpad[:].rearrange("p (r c) -> p r c", c=W)    # [128, 194, 192]

        # zero the two padding rows; column padding lives in the G buffer
        nc.vector.memset(x3[:, 0, :], 0.0)
        nc.vector.memset(x3[:, HP - 1, :], 0.0)
        nc.vector.memset(gbuf[:], 0.0)
        # stage lhsT tables are zero-padded to the full 128x128 array
        # (M=12 / K=12 matmuls leave HAM cold at 1.2 GHz; padding is free)
        nc.vector.memset(lkx[:], 0.0)
        nc.vector.memset(lky[:], 0.0)

        # input load via HWDGE fp32 into staging (SWDGE's engine-15
        # descriptor-ring contention made the cast-DMA tail ~9us slow);
        # ACT casts each chunk to bf16, DVE reduces the fp32 staging.
        h0 = 0
        for i, lr in enumerate(CHUNKS):
            slot = stag[:, (i % NSTAGE) * 16 * W:(i % NSTAGE) * 16 * W + lr * W]
            nc.sync.dma_start(out=slot, in_=xf[:, h0 * W:(h0 + lr) * W])
            nc.scalar.copy(out=x3[:, h0 + 1:h0 + 1 + lr, :], in_=slot)
            s3 = slot.rearrange("p (r c) -> p r c", c=W)
            nc.vector.reduce_sum(out=scr[:, 16 + i:17 + i], in_=slot, axis=AX.X)
            nc.vector.reduce_sum(
                out=scr[:, 16 + NCHUNK + i:17 + NCHUNK + i],
                in_=s3[:, :, 0], axis=AX.X)
            nc.vector.reduce_sum(
                out=scr[:, 16 + 2 * NCHUNK + i:17 + 2 * NCHUNK + i],
                in_=s3[:, :, W - 1], axis=AX.X)
            if i == 0:      # row-0 sum and top corners only need chunk 0
                nc.vector.reduce_sum(out=scr[:, 3:4], in_=slot[:, 0:W], axis=AX.X)
                nc.vector.tensor_copy(scr[:, 5:7], slot[:, 0:W:W - 1])
            if i == NCHUNK - 1:  # last-row sum and bottom corners
                nc.vector.reduce_sum(
                    out=scr[:, 4:5], in_=slot[:, (lr - 1) * W:lr * W], axis=AX.X)
                nc.vector.tensor_copy(
                    scr[:, 7:9], slot[:, (lr - 1) * W:lr * W:W - 1])
            h0 += lr

        # constants ride the gpsimd (SWDGE) queue, parallel to the input
        nc.gpsimd.dma_start(out=wkh[:], in_=wkh_d)
        nc.gpsimd.dma_start(out=lwin[:], in_=lwin_d)
        nc.gpsimd.dma_start(out=brep[:], in_=brep_d)
        nc.gpsimd.dma_start(out=wo9[:], in_=wo9_d)
        nc.gpsimd.dma_start(out=m12[:], in_=m12_d)
        nc.gpsimd.dma_start(out=kxm[:], in_=kxm_d)
        nc.gpsimd.dma_start(out=ident[:], in_=ident_d)

        # final sums: T/CF/CL in one grouped reduce
        nc.vector.reduce_sum(
            out=scr[:, 0:3],
            in_=scr[:, 16:16 + 3 * NCHUNK].rearrange("p (g i) -> p g i", g=3),
            axis=AX.X)

        # kernel[p, j] = sum_k wkH[p, j*9+k] * sums[p, k]
        sums9 = scr[:, 0:9].unsqueeze(1).broadcast_to([P, 9, 9])
        nc.vector.tensor_mul(t81[:].rearrange("p (j m) -> p j m", m=9), wkh[:].rearrange("p (j m) -> p j m", m=9), sums9)
        nc.vector.reduce_sum(
            out=kern[:], in_=t81[:].rearrange("p (j m) -> p j m", m=9), axis=AX.X)
        nc.vector.tensor_copy(kernb[:], kern[:])

        # win = blockdiag(w_in.T) @ kernel (+ b_in fused into the lky build)
        win_ps = psum_s.tile([P, 9], F32, tag="small")
        nc.tensor.matmul(win_ps[:], lhsT=lwin[:], rhs=kernb[:], start=True, stop=True)

        # stage1 weights: lky[(b,i),(b',kx)] = (win[b,i,3ky+kx]+b_in) d(b,b')
        m123 = m12[:].rearrange("p (b k) -> p b k", k=3)
        for ky in range(3):
            wv = win_ps[:, 3 * ky:3 * ky + 3].unsqueeze(1).broadcast_to([P, BC, 3])
            nc.vector.scalar_tensor_tensor(
                lky[:, ky * P:ky * P + 12].rearrange("p (b k) -> p b k", k=3),
                wv, brep[:], m123, op0=OP.add, op1=OP.mult)

        # wout[(b,o)] = sum_j blockdiag(w_out[:,:,j].T) @ kernel[:, j]
        wout_ps = psum_s.tile([P, 1], F32, tag="woutps")
        for j in range(9):
            nc.tensor.matmul(
                wout_ps[:], lhsT=wo9[:, j * P:(j + 1) * P], rhs=kernb[:, j:j + 1],
                start=(j == 0), stop=(j == 8))
        # stage2 weights: W12[(b,kx),(b',o)] = wout[b',o] d(b,b'), masked per kx
        nc.vector.tensor_scalar_mul(vout[:], m12[:], wout_ps[:, 0:1])
        w12_ps = psum_s.tile([12, P], F32, tag="small")
        nc.tensor.transpose(w12_ps[:], vout[:], ident[:])
        for kx in range(3):
            nc.vector.tensor_scalar_mul(
                lkx[0:12, kx * P:(kx + 1) * P], w12_ps[:], kxm[:, kx:kx + 1])

        # conv: stage1 (G) and stage2 (y), software-pipelined
        ysb_tiles = {}
        g4 = gbuf[:].rearrange("p (s r c) -> p s r c", s=3, c=WP)

        def stage1(t):
            h0 = t * R
            g_ps = psum_g.tile([P, R * W], F32, tag="gps")
            for ky in range(3):
                nc.tensor.matmul(
                    g_ps[:],
                    lhsT=lky[:, ky * P:(ky + 1) * P],
                    rhs=x3[:, h0 + ky:h0 + ky + R, :],
                    start=(ky == 0), stop=(ky == 2))
            # evict into padded G slot (side columns stay zero)
            nc.vector.tensor_copy(g4[:, t % 3, :, 1:1 + W], g_ps[:])

        def stage2(t):
            g = t // GT
            if g not in ysb_tiles:
                ysb_tiles[g] = ypool.tile(
                    [P, GT * R * W], F32, tag="ysb", name="ysb")
            ysb = ysb_tiles[g]
            y_ps = psum_y.tile([P, R * W], F32, tag="yps")
            for kx in range(3):
                nc.tensor.matmul(
                    y_ps[:],
                    lhsT=lkx[:, kx * P:(kx + 1) * P],
                    rhs=g4[:, t % 3, :, kx:kx + W],
                    start=(kx == 0), stop=(kx == 2))
            tt = t % GT
            nc.scalar.copy(out=ysb[:, tt * R * W:(tt + 1) * R * W], in_=y_ps[:])
            half = GT // 2
            if g == NG - 1 and tt == half - 1:
                nc.sync.dma_start(
                    out=yf[:, (g * GT) * R * W:(g * GT + half) * R * W],
                    in_=ysb[:, 0:half * R * W])
            elif g == NG - 1 and tt == GT - 1:
                nc.sync.dma_start(
                    out=yf[:, (g * GT + half) * R * W:(g + 1) * GT * R * W],
                    in_=ysb[:, half * R * W:])
            elif tt == GT - 1:
                nc.sync.dma_start(
                    out=yf[:, g * GT * R * W:(g + 1) * GT * R * W], in_=ysb[:])
                del ysb_tiles[g]

        stage1(0)
        for t in range(1, NT):
            stage1(t)
            stage2(t - 1)
        stage2(NT - 1)


def host_tables(wk, w_in, b_in, w_out):
    # H matrix: sums vector [T,CF,CL,RF,RL,c00,c0L,cL0,cLL] -> S[m], m=(dy,dx)
    Hm = np.zeros((9, 9), np.float32)
    Hm[0, :] = 1.0
    for m in range(9):
        dy, dx = divmod(m, 3)
        if dy == 0:
            Hm[4, m] -= 1.0
        if dy == 2:
            Hm[3, m] -= 1.0
        if dx == 0:
            Hm[2, m] -= 1.0
        if dx == 2:
            Hm[1, m] -= 1.0
    Hm[8, 0] = Hm[7, 2] = Hm[6, 6] = Hm[5, 8] = 1.0
    wk9 = wk.reshape(CIN, 9, 9).astype(np.float32) / float(H * W)  # [c, j, m]
    wkh = np.einsum("cjm,km->cjk", wk9, Hm).reshape(CIN, 81)
    wkh = np.tile(wkh, (BC, 1))

    lwin = np.kron(np.eye(BC, dtype=np.float32), w_in.T.astype(np.float32))
    brep = np.tile(b_in.astype(np.float32), BC)[:, None]
    w9 = w_out.reshape(COUT, CIN, 9).astype(np.float32)
    wo9 = np.concatenate(
        [np.kron(np.eye(BC, dtype=np.float32), w9[:, :, j].T) for j in range(9)],
        axis=1)
    # m12[(b~,i), (b,kx)] = d(b~==b)
    m12 = np.repeat(np.eye(BC, dtype=np.float32), CIN, axis=0)
    m12 = np.repeat(m12, 3, axis=1)  # [128, 12]
    # kxm[(b,kx'), kx] = d(kx'==kx)
    kxm = np.tile(np.eye(3, dtype=np.float32), (BC, 1))  # [12, 3]
    ident = np.eye(P, dtype=np.float32)
    return {
        "wkh": np.ascontiguousarray(wkh, np.float32),
        "lwin": np.ascontiguousarray(lwin).astype(ml_dtypes.bfloat16),
        "brep": np.ascontiguousarray(brep, np.float32),
        "wo9": np.ascontiguousarray(wo9).astype(ml_dtypes.bfloat16),
        "m12": np.ascontiguousarray(m12, np.float32),
        "kxm": np.ascontiguousarray(kxm, np.float32),
        "ident": np.ascontiguousarray(ident, np.float32),
    }


_CACHE: dict = {}


def _get_program() -> bass.Bass:
    if "nc" not in _CACHE:
        nc = bacc.Bacc(
            trn_type="TRN2", target_bir_lowering=False, debug=False,
            num_devices=NCORES)
        build_program(nc)
        nc.compile()
        _CACHE["nc"] = nc
    return _CACHE["nc"]


def kernel(x, wk, w_in, b_in, w_out, _trace=False, _trace_kwargs=None):
    x = np.ascontiguousarray(np.asarray(x), np.float32)
    tables = host_tables(np.asarray(wk), np.asarray(w_in), np.asarray(b_in),
                         np.asarray(w_out))
    nc = _get_program()
    in_maps = [
        {"x": np.ascontiguousarray(x[c * BC:(c + 1) * BC]), **tables}
        for c in range(NCORES)
    ]
    res = run_bass_kernel_spmd(
        nc, in_maps, core_ids=list(range(NCORES)),
        trace=_trace, **(_trace_kwargs or {}))
    y = np.concatenate([res.results[c]["y"] for c in range(NCORES)], axis=0)
    if _trace:
        return y, res
    return y


if __name__ == "__main__":
    rng = np.random.default_rng(0)
    inputs = {
        "x": rng.standard_normal((B, CIN, H, W), np.float32),
        "wk": rng.standard_normal((CIN * 9, 1, 3, 3)).astype(np.float32) * 0.05,
        "w_in": rng.standard_normal((CIN, CIN)).astype(np.float32) * 0.05,
        "b_in": rng.standard_normal((CIN,)).astype(np.float32) * 0.05,
        "w_out": rng.standard_normal((COUT, CIN, 3, 3)).astype(np.float32) * 0.05,
    }
    y = kernel(**inputs)
    print("y", y.shape, y.dtype, float(np.abs(y).max()))



# revision 6
# speedup vs baseline: 1.1152x; 1.1152x over previous
"""Trainium2 Bass kernel for nn_BaseConvPlus (dense_cnn).

Math: the reference computes
  1) kernel[b,c,:,:]  = global-mean of a depthwise 3x3 conv of x          -> [B,CIN,3,3]
  2) win  = einsum(kernel, w_in) + b_in ; wout = einsum(kernel, w_out)
  3) y[b] = conv2d(x[b], weight[b]) with weight[b,o,i] = win[b,i]*wout[b,o]

Identities / tricks:
  * mean(conv(x, k)) over HxW only needs the total sum, edge-row/col sums
    and corner pixels of each channel (zero 'SAME' padding) - no conv.
    The tap-selection matrix is folded into the host-side wk tables, so
    kernel[b,c,j] = sum_k wkH[c,j,k] * sums[b,c,k] with sums = the 9
    reduced quantities [T, CF, CL, RF, RL, c00, c0L, cL0, cLL].
  * weight[b] is rank-1 across (o, i): y[b,o] = wout[b,o] * z[b] with
    z[b] = sum_i conv2d(x[b,i], win[b,i]).
  * x arrives host-padded (zero ring) and bf16, so all 9 (ky,kx) taps are
    plain shifted windows of the padded image.  Stage 1 runs the 9 tap
    matmuls (M=4 real outputs each) as 3 accumulation chains on 3
    concurrent PE *column groups* (tile_position col packing), so the 9
    image passes cost ~3.  One [96,384] eviction per tile feeds stage 2,
    a single K=96 matmul that applies wout (1 pass).  ~4 effective
    passes/tile vs 6 in the two-stage K=128/K=12 formulation, and the
    eviction count drops 1.5x.
  * bf16 end to end (input pre-cast on host, output upcast on host):
    halves both HBM phases.  Dummy matmuls paced by the input chunks keep
    the PE HAM warm (2.4 GHz) through the load phase.

Sharding: pure data parallel, 4 samples per core on 8 cores.
"""
import sys

sys.path.insert(0, "/opt/trn_rl_repo")

from contextlib import ExitStack

import ml_dtypes
import numpy as np

import concourse.bacc as bacc
import concourse.bass as bass
import concourse.mybir as mybir
import concourse.tile as tile
from concourse.bass_utils import run_bass_kernel_spmd

B, CIN, COUT, KS, H, W = 32, 32, 32, 3, 192, 192
NCORES = 8
BC = B // NCORES          # 4 samples per core
P = BC * CIN              # 128 partitions = (sample, channel)
WP = W + 2                # 194 padded cols
HP = H + 2                # 194 padded rows
R = 2                     # output rows per conv tile
NT = H // R               # 96 conv tiles
GT = 8                    # conv tiles per output DMA (16 rows)
NG = NT // GT             # 12 output DMAs
N = R * W                 # 384 moving columns per matmul
CHUNKS = [15] * 12 + [10, 4]   # input chunks over the 194 padded rows
NCHUNK = len(CHUNKS)      # 14
F32 = mybir.dt.float32
BF16 = mybir.dt.bfloat16
AX = mybir.AxisListType
OP = mybir.AluOpType


def build_program(nc: bass.Bass) -> None:
    x_d = nc.dram_tensor("xpad", [P, HP * WP], BF16, kind="ExternalInput").ap()
    wkh_d = nc.dram_tensor("wkh", [P, 81], F32, kind="ExternalInput").ap()
    lwin_d = nc.dram_tensor("lwin", [P, P], BF16, kind="ExternalInput").ap()
    brep_d = nc.dram_tensor("brep", [P, 1], F32, kind="ExternalInput").ap()
    wo9_d = nc.dram_tensor("wo9", [P, 9 * P], BF16, kind="ExternalInput").ap()
    m4_d = nc.dram_tensor("m4", [P, 32], F32, kind="ExternalInput").ap()
    ident_d = nc.dram_tensor("ident", [P, P], F32, kind="ExternalInput").ap()
    y_d = nc.dram_tensor("y", [BC, COUT, H, W], BF16, kind="ExternalOutput").ap()

    yf = y_d.rearrange("b o h w -> (b o) (h w)")       # [128, 36864] bf16

    with tile.TileContext(nc) as tc, ExitStack() as ctx:
        const = ctx.enter_context(tc.tile_pool(name="const", bufs=1))
        ypool = ctx.enter_context(tc.tile_pool(name="ysb", bufs=2))
        psum_z = ctx.enter_context(tc.tile_pool(name="psum_z", bufs=3, space="PSUM"))
        psum_y = ctx.enter_context(tc.tile_pool(name="psum_y", bufs=2, space="PSUM"))
        psum_s = ctx.enter_context(tc.tile_pool(name="psum_s", bufs=1, space="PSUM"))
        psum_d = ctx.enter_context(tc.tile_pool(name="psum_d", bufs=1, space="PSUM"))

        xpad = const.tile([P, HP * WP], BF16)
        wkh = const.tile([P, 81], F32)
        lwin = const.tile([P, P], BF16)
        brep = const.tile([P, 1], F32)
        wo9 = const.tile([P, 9 * P], BF16)
        m4 = const.tile([P, 32], F32)
        ident = const.tile([P, P], F32)
        scr = const.tile([P, 16 + 3 * NCHUNK], F32)
        t81 = const.tile([P, 81], F32)
        kern = const.tile([P, 9], F32)
        kernb = const.tile([P, 9], BF16)
        wS1 = const.tile([P, 9 * 32], BF16)    # stage1 lhsT per (ky,kx)
        w2 = const.tile([96, P], BF16)         # stage2 lhsT (wout block diag)
        e4 = const.tile([P, 4], F32)
        gbuf = const.tile([96, 3 * N], BF16)   # 3 slots of stage1 output

        x3 = xpad[:].rearrange("p (r c) -> p r c", c=WP)   # [128, 194, 194]

        nc.vector.memset(w2[:], 0.0)

        # constants ride the gpsimd (SWDGE) queue, parallel to the input
        nc.gpsimd.dma_start(out=wkh[:], in_=wkh_d)
        nc.gpsimd.dma_start(out=lwin[:], in_=lwin_d)
        nc.gpsimd.dma_start(out=brep[:], in_=brep_d)
        nc.gpsimd.dma_start(out=wo9[:], in_=wo9_d)
        nc.gpsimd.dma_start(out=m4[:], in_=m4_d)
        nc.gpsimd.dma_start(out=ident[:], in_=ident_d)

        # PE keep-warm target (never read; WAW chain keeps it serialized)
        pdum = psum_d.tile([P, 512], F32, tag="dummy")

        # ---- input phase: chunked DMA + running sums + PE warmup ----
        r0 = 0
        for i, lr in enumerate(CHUNKS):
            sl = xpad[:, r0 * WP:(r0 + lr) * WP]
            nc.sync.dma_start(out=sl, in_=x_d[:, r0 * WP:(r0 + lr) * WP])
            s3 = sl.rearrange("p (r c) -> p r c", c=WP)
            nc.vector.reduce_sum(out=scr[:, 16 + i:17 + i], in_=sl, axis=AX.X)
            nc.vector.reduce_sum(
                out=scr[:, 16 + NCHUNK + i:17 + NCHUNK + i],
                in_=s3[:, :, 1], axis=AX.X)
            nc.vector.reduce_sum(
                out=scr[:, 16 + 2 * NCHUNK + i:17 + 2 * NCHUNK + i],
                in_=s3[:, :, W], axis=AX.X)
            if r0 <= 1 < r0 + lr:          # first data row + top corners
                nc.vector.reduce_sum(
                    out=scr[:, 3:4], in_=x3[:, 1, :], axis=AX.X)
                nc.vector.tensor_copy(
                    scr[:, 5:7], xpad[:, WP + 1:WP + W + 1:W - 1])
            if r0 <= H < r0 + lr:          # last data row + bottom corners
                nc.vector.reduce_sum(
                    out=scr[:, 4:5], in_=x3[:, H, :], axis=AX.X)
                nc.vector.tensor_copy(
                    scr[:, 7:9], xpad[:, H * WP + 1:H * WP + W + 1:W - 1])
            # HAM keep-warm: dummy matmuls paced by this chunk's arrival
            ndum = 8 if i < 12 else (4 if i == 12 else 3)
            for k in range(ndum):
                nc.tensor.matmul(
                    pdum[:], lhsT=xpad[:, 0:128],
                    rhs=xpad[:, r0 * WP:r0 * WP + 512],
                    start=True, stop=True)
            r0 += lr

        # final sums: T/CF/CL in one grouped reduce
        nc.vector.reduce_sum(
            out=scr[:, 0:3],
            in_=scr[:, 16:16 + 3 * NCHUNK].rearrange("p (g i) -> p g i", g=3),
            axis=AX.X)

        # kernel[p, j] = sum_k wkH[p, j*9+k] * sums[p, k]
        sums9 = scr[:, 0:9].unsqueeze(1).broadcast_to([P, 9, 9])
        nc.vector.tensor_mul(
            t81[:].rearrange("p (j m) -> p j m", m=9),
            wkh[:].rearrange("p (j m) -> p j m", m=9), sums9)
        nc.vector.reduce_sum(
            out=kern[:], in_=t81[:].rearrange("p (j m) -> p j m", m=9),
            axis=AX.X)
        nc.vector.tensor_copy(kernb[:], kern[:])

        # one shared PSUM bank for the small weight-build outputs
        small = psum_s.tile([P, 160], F32, tag="small")
        win_ps = small[:, 0:9]
        wout_ps = small[:, 9:10]
        et_ps = small[0:4, 32:160]

        # win = blockdiag(w_in.T) @ kernel  (+ b_in added in the stt below)
        nc.tensor.matmul(win_ps, lhsT=lwin[:], rhs=kernb[:],
                         start=True, stop=True)

        # stage1 weights: wS1[(b,i), (j,b')] = (win[b,i,j]+b_in[i]) d(b,b')
        wv = win_ps.unsqueeze(2).broadcast_to([P, 9, 32])
        mv = m4[:].unsqueeze(1).broadcast_to([P, 9, 32])
        nc.vector.scalar_tensor_tensor(
            wS1[:].rearrange("p (j c) -> p j c", c=32),
            wv, brep[:], mv, op0=OP.add, op1=OP.mult)

        # wout[(b,o)] = sum_j blockdiag(w_out[:,:,j].T) @ kernel[:, j]
        for j in range(9):
            nc.tensor.matmul(
                wout_ps, lhsT=wo9[:, j * P:(j + 1) * P],
                rhs=kernb[:, j:j + 1], start=(j == 0), stop=(j == 8))
        # W2[32g+b, (b',o)] = wout[(b',o)] d(b,b') for g=0,1,2
        nc.vector.tensor_scalar_mul(e4[:], m4[:, 0:4], wout_ps)
        nc.tensor.transpose(et_ps, e4[:], ident[:])
        for g in range(3):
            nc.vector.tensor_copy(w2[32 * g:32 * g + 4, :], et_ps)

        # ---- conv phase ----
        # (gpsimd/Pool cannot read PSUM: evictions alternate DVE <-> ACT)
        ysb_tiles = {}
        zcp = [nc.vector.tensor_copy,
               lambda o, i_: nc.scalar.copy(out=o, in_=i_)]

        def stage1(t):
            z_ps = psum_z.tile([96, N], F32, tag="zps")
            for ky in range(3):
                for g in range(3):
                    j = 3 * ky + g
                    nc.tensor.matmul(
                        z_ps[32 * g:32 * g + 32, :],
                        lhsT=wS1[:, j * 32:(j + 1) * 32],
                        rhs=x3[:, 2 * t + ky:2 * t + ky + 2, g:g + W],
                        start=(ky == 0), stop=(ky == 2))
            zcp[t % 2](gbuf[:, (t % 3) * N:(t % 3 + 1) * N], z_ps[:])

        def stage2(t):
            g = t // GT
            if g not in ysb_tiles:
                ysb_tiles[g] = ypool.tile([P, GT * N], BF16, tag="ysb",
                                          name="ysb")
            ysb = ysb_tiles[g]
            y_ps = psum_y.tile([P, N], F32, tag="yps")
            nc.tensor.matmul(y_ps[:], lhsT=w2[:], rhs=gbuf[:, (t % 3) * N:(t % 3 + 1) * N],
                             start=True, stop=True)
            tt = t % GT
            zcp[(t + 1) % 2](ysb[:, tt * N:(tt + 1) * N], y_ps[:])
            half = GT // 2
            if g == NG - 1 and tt == half - 1:
                nc.sync.dma_start(
                    out=yf[:, (g * GT) * N:(g * GT + half) * N],
                    in_=ysb[:, 0:half * N])
            elif g == NG - 1 and tt == GT - 1:
                nc.sync.dma_start(
                    out=yf[:, (g * GT + half) * N:(g + 1) * GT * N],
                    in_=ysb[:, half * N:])
            elif tt == GT - 1:
                nc.sync.dma_start(
                    out=yf[:, g * GT * N:(g + 1) * GT * N], in_=ysb[:])
                del ysb_tiles[g]

        stage1(0)
        stage1(1)
        for t in range(NT):
            if t + 2 < NT:
                stage1(t + 2)
            stage2(t)


def host_tables(wk, w_in, b_in, w_out):
    # H matrix: sums vector [T,CF,CL,RF,RL,c00,c0L,cL0,cLL] -> S[m], m=(dy,dx)
    Hm = np.zeros((9, 9), np.float32)
    Hm[0, :] = 1.0
    for m in range(9):
        dy, dx = divmod(m, 3)
        if dy == 0:
            Hm[4, m] -= 1.0
        if dy == 2:
            Hm[3, m] -= 1.0
        if dx == 0:
            Hm[2, m] -= 1.0
        if dx == 2:
            Hm[1, m] -= 1.0
    Hm[8, 0] = Hm[7, 2] = Hm[6, 6] = Hm[5, 8] = 1.0
    wk9 = wk.reshape(CIN, 9, 9).astype(np.float32) / float(H * W)  # [c, j, m]
    wkh = np.einsum("cjm,km->cjk", wk9, Hm).reshape(CIN, 81)
    wkh = np.tile(wkh, (BC, 1))

    lwin = np.kron(np.eye(BC, dtype=np.float32), w_in.T.astype(np.float32))
    brep = np.tile(b_in.astype(np.float32), BC)[:, None]
    w9 = w_out.reshape(COUT, CIN, 9).astype(np.float32)
    wo9 = np.concatenate(
        [np.kron(np.eye(BC, dtype=np.float32), w9[:, :, j].T) for j in range(9)],
        axis=1)
    m4 = np.zeros((P, 32), np.float32)
    m4[np.arange(P), np.arange(P) // CIN] = 1.0
    ident = np.eye(P, dtype=np.float32)
    return {
        "wkh": np.ascontiguousarray(wkh, np.float32),
        "lwin": np.ascontiguousarray(lwin).astype(ml_dtypes.bfloat16),
        "brep": np.ascontiguousarray(brep, np.float32),
        "wo9": np.ascontiguousarray(wo9).astype(ml_dtypes.bfloat16),
        "m4": np.ascontiguousarray(m4, np.float32),
        "ident": np.ascontiguousarray(ident, np.float32),
    }


_CACHE: dict = {}


def _get_program() -> bass.Bass:
    if "nc" not in _CACHE:
        nc = bacc.Bacc(
            trn_type="TRN2", target_bir_lowering=False, debug=False,
            num_devices=NCORES)
        build_program(nc)
        nc.compile()
        _CACHE["nc"] = nc
    return _CACHE["nc"]


def kernel(x, wk, w_in, b_in, w_out, _trace=False, _trace_kwargs=None):
    x = np.asarray(x, np.float32)
    xp = np.zeros((B, CIN, HP, WP), np.float32)
    xp[:, :, 1:H + 1, 1:W + 1] = x
    xpb = xp.astype(ml_dtypes.bfloat16).reshape(B, CIN, HP * WP)
    tables = host_tables(np.asarray(wk), np.asarray(w_in), np.asarray(b_in),
                         np.asarray(w_out))
    nc = _get_program()
    in_maps = [
        {"xpad": np.ascontiguousarray(
            xpb[c * BC:(c + 1) * BC].reshape(P, HP * WP)), **tables}
        for c in range(NCORES)
    ]
    res = run_bass_kernel_spmd(
        nc, in_maps, core_ids=list(range(NCORES)),
        trace=_trace, **(_trace_kwargs or {}))
    y = np.concatenate(
        [np.asarray(res.results[c]["y"]).astype(np.float32)
         for c in range(NCORES)], axis=0)
    if _trace:
        return y, res
    return y


if __name__ == "__main__":
    rng = np.random.default_rng(0)
    inputs = {
        "x": rng.standard_normal((B, CIN, H, W), np.float32),
        "wk": rng.standard_normal((CIN * 9, 1, 3, 3)).astype(np.float32) * 0.05,
        "w_in": rng.standard_normal((CIN, CIN)).astype(np.float32) * 0.05,
        "b_in": rng.standard_normal((CIN,)).astype(np.float32) * 0.05,
        "w_out": rng.standard_normal((COUT, CIN, 3, 3)).astype(np.float32) * 0.05,
    }
    y = kernel(**inputs)
    print("y", y.shape, y.dtype, float(np.abs(y).max()))


# revision 10
# speedup vs baseline: 1.4611x; 1.3102x over previous
"""Trainium2 Bass kernel for nn_BaseConvPlus (dense_cnn).

Math: the reference computes
  1) kernel[b,c,:,:]  = global-mean of a depthwise 3x3 conv of x          -> [B,CIN,3,3]
  2) win  = einsum(kernel, w_in) + b_in ; wout = einsum(kernel, w_out)
  3) y[b] = conv2d(x[b], weight[b]) with weight[b,o,i] = win[b,i]*wout[b,o]

Identities / tricks:
  * mean(conv(x, k)) over HxW only needs the total sum, edge-row/col sums
    and corner pixels of each channel (zero 'SAME' padding) - no conv.
    The tap-selection matrix is folded into the host-side wk tables, so
    kernel[b,c,j] = sum_k wkH[c,j,k] * sums[b,c,k] with sums = the 9
    reduced quantities [T, CF, CL, RF, RL, c00, c0L, cL0, cLL].
  * weight[b] is rank-1 across (o, i): y[b,o] = wout[b,o] * z[b] with
    z[b] = sum_i conv2d(x[b,i], win[b,i]).
  * x arrives host-padded (zero ring) and bf16, so all 9 (ky,kx) taps are
    plain shifted windows of the padded image.  Stage 1 runs the 9 tap
    matmuls (M=4 real outputs each) as 3 accumulation chains on 3
    concurrent PE *column groups* (tile_position col packing), so the 9
    image passes cost ~3.  One [96,384] eviction per tile feeds stage 2,
    a single K=96 matmul that applies wout (1 pass).  ~4 effective
    passes/tile vs 6 in the two-stage K=128/K=12 formulation, and the
    eviction count drops 1.5x.
  * bf16 end to end (input pre-cast on host, output upcast on host):
    halves both HBM phases.  Dummy matmuls paced by the input chunks keep
    the PE HAM warm (2.4 GHz) through the load phase.

Sharding: pure data parallel, 4 samples per core on 8 cores.
"""
import sys

sys.path.insert(0, "/opt/trn_rl_repo")

from contextlib import ExitStack

import ml_dtypes
import numpy as np

import concourse.bacc as bacc
import concourse.bass as bass
import concourse.mybir as mybir
import concourse.tile as tile
from concourse.bass_utils import run_bass_kernel_spmd

B, CIN, COUT, KS, H, W = 32, 32, 32, 3, 192, 192
NCORES = 8
BC = B // NCORES          # 4 samples per core
P = BC * CIN              # 128 partitions = (sample, channel)
WP = W + 2                # 194 padded cols
HP = H + 2                # 194 padded rows
R = 2                     # output rows per conv tile
NT = H // R               # 96 conv tiles
GT = 8                    # conv tiles per output DMA (16 rows)
NG = NT // GT             # 12 output DMAs
N = R * W                 # 384 moving columns per matmul
CHUNKS = [15] * 12 + [10, 4]   # input chunks over the 194 padded rows
NCHUNK = len(CHUNKS)      # 14
F32 = mybir.dt.float32
BF16 = mybir.dt.bfloat16
AX = mybir.AxisListType
OP = mybir.AluOpType


def build_program(nc: bass.Bass) -> None:
    x_d = nc.dram_tensor("xpad", [P, HP * WP], BF16, kind="ExternalInput").ap()
    wkh_d = nc.dram_tensor("wkh", [P, 81], F32, kind="ExternalInput").ap()
    lwin_d = nc.dram_tensor("lwin", [P, P], BF16, kind="ExternalInput").ap()
    brep_d = nc.dram_tensor("brep", [P, 1], F32, kind="ExternalInput").ap()
    wo9_d = nc.dram_tensor("wo9", [P, 9 * P], BF16, kind="ExternalInput").ap()
    m4_d = nc.dram_tensor("m4", [P, 32], F32, kind="ExternalInput").ap()
    ident_d = nc.dram_tensor("ident", [P, P], F32, kind="ExternalInput").ap()
    y_d = nc.dram_tensor("y", [BC, COUT, H, W], BF16, kind="ExternalOutput").ap()

    yf = y_d.rearrange("b o h w -> (b o) (h w)")       # [128, 36864] bf16

    with tile.TileContext(nc) as tc, ExitStack() as ctx:
        const = ctx.enter_context(tc.tile_pool(name="const", bufs=1))
        ypool = ctx.enter_context(tc.tile_pool(name="ysb", bufs=2))
        psum_z = ctx.enter_context(tc.tile_pool(name="psum_z", bufs=3, space="PSUM"))
        psum_y = ctx.enter_context(tc.tile_pool(name="psum_y", bufs=2, space="PSUM"))
        psum_s = ctx.enter_context(tc.tile_pool(name="psum_s", bufs=1, space="PSUM"))
        psum_d = ctx.enter_context(tc.tile_pool(name="psum_d", bufs=1, space="PSUM"))

        xpad = const.tile([P, HP * WP], BF16)
        wkh = const.tile([P, 81], F32)
        lwin = const.tile([P, P], BF16)
        brep = const.tile([P, 1], F32)
        wo9 = const.tile([P, 9 * P], BF16)
        m4 = const.tile([P, 32], F32)
        ident = const.tile([P, P], F32)
        scr = const.tile([P, 16 + 3 * NCHUNK], F32)
        t81 = const.tile([P, 81], F32)
        kern = const.tile([P, 9], F32)
        kernb = const.tile([P, 9], BF16)
        wS1 = const.tile([P, 9 * 32], BF16)    # stage1 lhsT per (ky,kx)
        w2 = const.tile([96, P], BF16)         # stage2 lhsT (wout block diag)
        e4 = const.tile([P, 4], F32)
        gbuf = const.tile([96, 3 * N], BF16)   # 3 slots of stage1 output
        rscr = const.tile([P, 15 * WP], BF16)  # ACT accum-reduce trash output

        x3 = xpad[:].rearrange("p (r c) -> p r c", c=WP)   # [128, 194, 194]

        nc.vector.memset(w2[:], 0.0)

        # constants ride the gpsimd (SWDGE) queue, parallel to the input
        nc.gpsimd.dma_start(out=wkh[:], in_=wkh_d)
        nc.gpsimd.dma_start(out=lwin[:], in_=lwin_d)
        nc.gpsimd.dma_start(out=brep[:], in_=brep_d)
        nc.gpsimd.dma_start(out=wo9[:], in_=wo9_d)
        nc.gpsimd.dma_start(out=m4[:], in_=m4_d)
        nc.gpsimd.dma_start(out=ident[:], in_=ident_d)

        # PE keep-warm target (never read; WAW chain keeps it serialized)
        pdum = psum_d.tile([P, 512], F32, tag="dummy")

        # ---- input phase: chunked DMA + running sums + PE warmup ----
        r0 = 0
        for i, lr in enumerate(CHUNKS):
            sl = xpad[:, r0 * WP:(r0 + lr) * WP]
            nc.sync.dma_start(out=sl, in_=x_d[:, r0 * WP:(r0 + lr) * WP])
            s3 = sl.rearrange("p (r c) -> p r c", c=WP)
            # big row-sum reduces alternate DVE <-> ACT (accum_out trick)
            # so neither engine falls behind the chunk arrival rate
            if i % 2 == 0:
                nc.vector.reduce_sum(out=scr[:, 16 + i:17 + i], in_=sl,
                                     axis=AX.X)
            else:
                nc.scalar.activation(
                    out=rscr[:, 0:lr * WP], in_=sl,
                    func=mybir.ActivationFunctionType.Copy,
                    accum_out=scr[:, 16 + i:17 + i])
            nc.vector.reduce_sum(
                out=scr[:, 16 + NCHUNK + i:17 + NCHUNK + i],
                in_=s3[:, :, 1], axis=AX.X)
            nc.vector.reduce_sum(
                out=scr[:, 16 + 2 * NCHUNK + i:17 + 2 * NCHUNK + i],
                in_=s3[:, :, W], axis=AX.X)
            if r0 <= 1 < r0 + lr:          # first data row + top corners
                nc.vector.reduce_sum(
                    out=scr[:, 3:4], in_=x3[:, 1, :], axis=AX.X)
                nc.vector.tensor_copy(
                    scr[:, 5:7], xpad[:, WP + 1:WP + W + 1:W - 1])
            if r0 <= H < r0 + lr:          # last data row + bottom corners
                nc.vector.reduce_sum(
                    out=scr[:, 4:5], in_=x3[:, H, :], axis=AX.X)
                nc.vector.tensor_copy(
                    scr[:, 7:9], xpad[:, H * WP + 1:H * WP + W + 1:W - 1])
            # HAM keep-warm: dummy matmuls paced by this chunk's arrival
            ndum = 6 if i < 12 else (3 if i == 12 else 2)
            for k in range(ndum):
                nc.tensor.matmul(
                    pdum[:], lhsT=xpad[:, 0:128],
                    rhs=xpad[:, r0 * WP:r0 * WP + 512],
                    start=True, stop=True)
            r0 += lr

        # final sums: T/CF/CL in one grouped reduce
        nc.vector.reduce_sum(
            out=scr[:, 0:3],
            in_=scr[:, 16:16 + 3 * NCHUNK].rearrange("p (g i) -> p g i", g=3),
            axis=AX.X)

        # kernel[p, j] = sum_k wkH[p, j*9+k] * sums[p, k]
        sums9 = scr[:, 0:9].unsqueeze(1).broadcast_to([P, 9, 9])
        nc.vector.tensor_mul(
            t81[:].rearrange("p (j m) -> p j m", m=9),
            wkh[:].rearrange("p (j m) -> p j m", m=9), sums9)
        nc.vector.reduce_sum(
            out=kern[:], in_=t81[:].rearrange("p (j m) -> p j m", m=9),
            axis=AX.X)
        nc.vector.tensor_copy(kernb[:], kern[:])

        # one shared PSUM bank for the small weight-build outputs
        small = psum_s.tile([P, 160], F32, tag="small")
        win_ps = small[:, 0:9]
        wout_ps = small[:, 9:10]
        et_ps = small[0:4, 32:160]

        # win = blockdiag(w_in.T) @ kernel  (+ b_in added in the stt below)
        nc.tensor.matmul(win_ps, lhsT=lwin[:], rhs=kernb[:],
                         start=True, stop=True)

        # stage1 weights: wS1[(b,i), (j,b')] = (win[b,i,j]+b_in[i]) d(b,b')
        wv = win_ps.unsqueeze(2).broadcast_to([P, 9, 32])
        mv = m4[:].unsqueeze(1).broadcast_to([P, 9, 32])
        nc.vector.scalar_tensor_tensor(
            wS1[:].rearrange("p (j c) -> p j c", c=32),
            wv, brep[:], mv, op0=OP.add, op1=OP.mult)

        # wout[(b,o)] = sum_j blockdiag(w_out[:,:,j].T) @ kernel[:, j]
        for j in range(9):
            nc.tensor.matmul(
                wout_ps, lhsT=wo9[:, j * P:(j + 1) * P],
                rhs=kernb[:, j:j + 1], start=(j == 0), stop=(j == 8))
        # W2[32g+b, (b',o)] = wout[(b',o)] d(b,b') for g=0,1,2
        nc.vector.tensor_scalar_mul(e4[:], m4[:, 0:4], wout_ps)
        nc.tensor.transpose(et_ps, e4[:], ident[:])
        for g in range(3):
            nc.vector.tensor_copy(w2[32 * g:32 * g + 4, :], et_ps)

        # ---- conv phase ----
        # (gpsimd/Pool cannot read PSUM: evictions alternate DVE <-> ACT)
        ysb_tiles = {}
        zcp = [nc.vector.tensor_copy,
               lambda o, i_: nc.scalar.copy(out=o, in_=i_)]

        def stage1(t):
            z_ps = psum_z.tile([96, N], F32, tag="zps")
            for ky in range(3):
                for g in range(3):
                    j = 3 * ky + g
                    nc.tensor.matmul(
                        z_ps[32 * g:32 * g + 32, :],
                        lhsT=wS1[:, j * 32:(j + 1) * 32],
                        rhs=x3[:, 2 * t + ky:2 * t + ky + 2, g:g + W],
                        start=(ky == 0), stop=(ky == 2))
            zcp[t % 2](gbuf[:, (t % 3) * N:(t % 3 + 1) * N], z_ps[:])

        def stage2(t):
            g = t // GT
            if g not in ysb_tiles:
                ysb_tiles[g] = ypool.tile([P, GT * N], BF16, tag="ysb",
                                          name="ysb")
            ysb = ysb_tiles[g]
            y_ps = psum_y.tile([P, N], F32, tag="yps")
            for g2 in range(4):
                nc.tensor.matmul(
                    y_ps[32 * g2:32 * g2 + 32, :],
                    lhsT=w2[:, 32 * g2:32 * g2 + 32],
                    rhs=gbuf[:, (t % 3) * N:(t % 3 + 1) * N],
                    start=True, stop=True,
                    tile_position=(0, 32 * g2))
            tt = t % GT
            zcp[(t + 1) % 2](ysb[:, tt * N:(tt + 1) * N], y_ps[:])
            half = GT // 2
            if g == NG - 1 and tt == half - 1:
                nc.sync.dma_start(
                    out=yf[:, (g * GT) * N:(g * GT + half) * N],
                    in_=ysb[:, 0:half * N])
            elif g == NG - 1 and tt == GT - 1:
                nc.sync.dma_start(
                    out=yf[:, (g * GT + half) * N:(g + 1) * GT * N],
                    in_=ysb[:, half * N:])
            elif tt == GT - 1:
                nc.sync.dma_start(
                    out=yf[:, g * GT * N:(g + 1) * GT * N], in_=ysb[:])
                del ysb_tiles[g]

        stage1(0)
        stage1(1)
        for t in range(NT):
            if t + 2 < NT:
                stage1(t + 2)
            stage2(t)


def host_tables(wk, w_in, b_in, w_out):
    # H matrix: sums vector [T,CF,CL,RF,RL,c00,c0L,cL0,cLL] -> S[m], m=(dy,dx)
    Hm = np.zeros((9, 9), np.float32)
    Hm[0, :] = 1.0
    for m in range(9):
        dy, dx = divmod(m, 3)
        if dy == 0:
            Hm[4, m] -= 1.0
        if dy == 2:
            Hm[3, m] -= 1.0
        if dx == 0:
            Hm[2, m] -= 1.0
        if dx == 2:
            Hm[1, m] -= 1.0
    Hm[8, 0] = Hm[7, 2] = Hm[6, 6] = Hm[5, 8] = 1.0
    wk9 = wk.reshape(CIN, 9, 9).astype(np.float32) / float(H * W)  # [c, j, m]
    wkh = np.einsum("cjm,km->cjk", wk9, Hm).reshape(CIN, 81)
    wkh = np.tile(wkh, (BC, 1))

    lwin = np.kron(np.eye(BC, dtype=np.float32), w_in.T.astype(np.float32))
    brep = np.tile(b_in.astype(np.float32), BC)[:, None]
    w9 = w_out.reshape(COUT, CIN, 9).astype(np.float32)
    wo9 = np.concatenate(
        [np.kron(np.eye(BC, dtype=np.float32), w9[:, :, j].T) for j in range(9)],
        axis=1)
    m4 = np.zeros((P, 32), np.float32)
    m4[np.arange(P), np.arange(P) // CIN] = 1.0
    ident = np.eye(P, dtype=np.float32)
    return {
        "wkh": np.ascontiguousarray(wkh, np.float32),
        "lwin": np.ascontiguousarray(lwin).astype(ml_dtypes.bfloat16),
        "brep": np.ascontiguousarray(brep, np.float32),
        "wo9": np.ascontiguousarray(wo9).astype(ml_dtypes.bfloat16),
        "m4": np.ascontiguousarray(m4, np.float32),
        "ident": np.ascontiguousarray(ident, np.float32),
    }


_CACHE: dict = {}


def _get_program() -> bass.Bass:
    if "nc" not in _CACHE:
        nc = bacc.Bacc(
            trn_type="TRN2", target_bir_lowering=False, debug=False,
            num_devices=NCORES)
        build_program(nc)
        nc.compile()
        _CACHE["nc"] = nc
    return _CACHE["nc"]


def kernel(x, wk, w_in, b_in, w_out, _trace=False, _trace_kwargs=None):
    x = np.asarray(x, np.float32)
    xp = np.zeros((B, CIN, HP, WP), np.float32)
    xp[:, :, 1:H + 1, 1:W + 1] = x
    xpb = xp.astype(ml_dtypes.bfloat16).reshape(B, CIN, HP * WP)
    tables = host_tables(np.asarray(wk), np.asarray(w_in), np.asarray(b_in),
                         np.asarray(w_out))
    nc = _get_program()
    in_maps = [
        {"xpad": np.ascontiguousarray(
            xpb[c * BC:(c + 1) * BC].reshape(P, HP * WP)), **tables}
        for c in range(NCORES)
    ]
    res = run_bass_kernel_spmd(
        nc, in_maps, core_ids=list(range(NCORES)),
        trace=_trace, **(_trace_kwargs or {}))
    y = np.concatenate(
        [np.asarray(res.results[c]["y"]).astype(np.float32)
         for c in range(NCORES)], axis=0)
    if _trace:
        return y, res
    return y


if __name__ == "__main__":
    rng = np.random.default_rng(0)
    inputs = {
        "x": rng.standard_normal((B, CIN, H, W), np.float32),
        "wk": rng.standard_normal((CIN * 9, 1, 3, 3)).astype(np.float32) * 0.05,
        "w_in": rng.standard_normal((CIN, CIN)).astype(np.float32) * 0.05,
        "b_in": rng.standard_normal((CIN,)).astype(np.float32) * 0.05,
        "w_out": rng.standard_normal((COUT, CIN, 3, 3)).astype(np.float32) * 0.05,
    }
    y = kernel(**inputs)
    print("y", y.shape, y.dtype, float(np.abs(y).max()))
